# revision 8
# baseline (speedup 1.0000x reference)
"""Trainium2 Bass kernel for GTStepwiseConstantVelocityModel.

Circulant pairing over the 384 nodes: core c owns the 48 contiguous nodes
[48c, 48c+48) and computes pairs (n, n+d mod 384) for d=1..191, plus a
24-pair slice of the d=192 strip.  Layout: t (=128) on partitions,
(d, k) pair columns on the free axis, processed in 8 chunks of <=1152
columns.  All hot element-wise traffic is fp16 so the DVE runs its 2x_1p
perf mode; the per-pair diffs are produced by single tensor_tensor ops
with overlapping/broadcast access patterns; step positions are cumsum'd
on the host and shipped as fp16 inputs.  ACT work is grouped into table
phases (Square+Ln | Exp | Erf) in two chunk-batches, with token columns
forcing the scheduler to honour the grouping (5 activation-table loads
total), so batch B's Square/Ln phase overlaps batch A's Exp phase.
m = (dzx*dvy - dzy*dvx)^2 / |dv|^2 replaces r2 - b^2/a2 (no cancellation
in fp16).  The t4 product is fused with the column reduction via
scalar_tensor_tensor accum_out (fp32).  Event term: host-built one-hot
gather matmuls.  Each core emits [event_partial, nonevent_partial]; the
host sums the 8 pairs.
"""
import numpy as np

N, D, T, E, NC = 384, 2, 128, 256, 8
NROW = 48            # nodes per core
EV_PER = E // NC     # 32 events per core
NDCOL = 240          # rotation columns per component
STRW = 24            # strip pairs per core
VW = 288             # per-component input width (240 + 24 + 24)
W = 1152             # full chunk width (24 d-values x 48 nodes)
NCH = 8              # 7 chunks of 24 d's + 1 chunk of 23 d's + strip
EPS_LN = 1e-6
LN_SPI2 = float(np.log(np.sqrt(np.pi) / 2.0))

_CACHE = {}


def _build_program(dt):
    from contextlib import ExitStack
    import concourse.bacc as bacc
    import concourse.tile as tile
    import concourse.mybir as mybir

    f32 = mybir.dt.float32
    f16 = mybir.dt.float16
    AF = mybir.ActivationFunctionType
    OP = mybir.AluOpType
    AX = mybir.AxisListType

    nc = bacc.Bacc("TRN2", target_bir_lowering=False, debug=False, num_devices=NC)

    def din(name, shape, dt_=f32):
        return nc.dram_tensor(name, shape, dt_, kind="ExternalInput").ap()

    vzxy_d = din("vzxy", [T, 4 * VW], f16)
    vnatx_d = din("vnatx", [N, T], f16)
    vnaty_d = din("vnaty", [N, T], f16)
    qmat_d = din("qmat", [N, EV_PER], f16)
    wmat_d = din("wmat", [T, EV_PER])
    bhot_d = din("bhot", [T, EV_PER])
    dz0x_d = din("dz0x", [1, EV_PER])
    dz0y_d = din("dz0y", [1, EV_PER])
    betac_d = din("betac", [T, 1])
    out_d = nc.dram_tensor("out", [1, 2], f32, kind="ExternalOutput").ap()

    with ExitStack() as ctx:
        tc = ctx.enter_context(tile.TileContext(nc))
        sg = ctx.enter_context(tc.tile_pool(name="singles", bufs=1))
        wk = ctx.enter_context(tc.tile_pool(name="work", bufs=2))
        ps = ctx.enter_context(tc.tile_pool(name="psum", bufs=1, space="PSUM"))

        def load(dram, shape, tag, dt_=f32):
            t = sg.tile(shape, dt_, name=tag, tag=tag)
            nc.sync.dma_start(out=t[:], in_=dram[:])
            return t

        vzxy = load(vzxy_d, [T, 4 * VW], "vzxy", f16)
        vxy = vzxy[:, 0:2 * VW]
        zxy = vzxy[:, 2 * VW:4 * VW]
        vnx = [load(vnatx_d[128 * r:128 * (r + 1), :], [128, T], f"vnx{r}", f16) for r in range(3)]
        vny = [load(vnaty_d[128 * r:128 * (r + 1), :], [128, T], f"vny{r}", f16) for r in range(3)]
        qm = [load(qmat_d[128 * r:128 * (r + 1), :], [128, EV_PER], f"qm{r}", f16) for r in range(3)]
        wmat = load(wmat_d, [T, EV_PER], "wmat")
        bhot = load(bhot_d, [T, EV_PER], "bhot")
        dz0x = load(dz0x_d, [1, EV_PER], "dz0x")
        dz0y = load(dz0y_d, [1, EV_PER], "dz0y")
        betac = load(betac_d, [T, 1], "betac")

        ones = sg.tile([T, 1], f32, name="ones")
        nc.vector.memset(ones[:], 1.0)
        epscol = sg.tile([T, 1], f32, name="epscol")
        nc.vector.memset(epscol[:], EPS_LN)
        lndtcol = sg.tile([T, 1], f32, name="lndtcol")
        nc.vector.memset(lndtcol[:], float(np.log(dt)))
        warm = sg.tile([T, 1], f32, name="warm")
        nc.vector.memset(warm[:], 1.0)
        bln = sg.tile([T, 1], f32, name="bln")
        nc.vector.tensor_scalar_add(out=bln[:], in0=betac[:], scalar1=LN_SPI2)

        # ---- event term ----
        gdx = ps.tile([T, EV_PER], f32, name="gdx")
        gdy = ps.tile([T, EV_PER], f32, name="gdy")
        for r in range(3):
            nc.tensor.matmul(gdx[:], vnx[r][:], qm[r][:], start=(r == 0), stop=(r == 2))
        for r in range(3):
            nc.tensor.matmul(gdy[:], vny[r][:], qm[r][:], start=(r == 0), stop=(r == 2))
        hx = sg.tile([T, EV_PER], f32, name="hx")
        hy = sg.tile([T, EV_PER], f32, name="hy")
        nc.vector.tensor_mul(hx[:], gdx[:], wmat[:])
        nc.vector.tensor_mul(hy[:], gdy[:], wmat[:])
        shx = ps.tile([1, EV_PER], f32, name="shx")
        shy = ps.tile([1, EV_PER], f32, name="shy")
        brow = ps.tile([1, EV_PER], f32, name="brow")
        nc.tensor.matmul(shx[:], ones[:], hx[:])
        nc.tensor.matmul(shy[:], ones[:], hy[:])
        nc.tensor.matmul(brow[:], betac[:], bhot[:])
        evx = sg.tile([1, EV_PER], f32, name="evx")
        evy = sg.tile([1, EV_PER], f32, name="evy")
        nc.vector.tensor_add(evx[:], shx[:], dz0x[:])
        nc.vector.tensor_add(evy[:], shy[:], dz0y[:])
        nc.vector.tensor_mul(evx[:], evx[:], evx[:])
        nc.vector.tensor_mul(evy[:], evy[:], evy[:])
        nc.vector.tensor_add(evx[:], evx[:], evy[:])
        evel = sg.tile([1, EV_PER], f32, name="evel")
        nc.vector.tensor_sub(evel[:], brow[:], evx[:])
        ev_s = sg.tile([1, 1], f32, name="ev_s")
        nc.vector.reduce_sum(out=ev_s[:], in_=evel[:], axis=AX.X)

        # warm up the natural_log table before the first Square lands
        warmout = sg.tile([T, 1], f32, name="warmout")
        nc.scalar.activation(warmout[:], warm[:], AF.Ln)

        def view2(t, wc, half=W):
            v = t[:, 0:1].copy()
            v.ap[1] = [half, 2]
            v.ap.append([1, wc])
            return v

        def ovdiff(out_tile, src_tile, d0, nd):
            in0 = src_tile[:, d0:d0 + 1].copy()
            in0.ap[1] = [VW, 2]
            in0.ap.append([1, nd])
            in0.ap.append([1, NROW])
            in1 = src_tile[:, 0:1].copy()
            in1.ap[1] = [VW, 2]
            in1.ap.append([0, nd])
            in1.ap.append([1, NROW])
            o = out_tile[:, 0:1].copy()
            o.ap[1] = [W, 2]
            o.ap.append([NROW, nd])
            o.ap.append([1, NROW])
            nc.vector.tensor_tensor(out=o, in0=in0, in1=in1, op=OP.subtract)

        def stripdiff(out_tile, src_tile, at):
            in0 = src_tile[:, NDCOL:NDCOL + 1].copy()
            in0.ap[1] = [VW, 2]
            in0.ap.append([1, STRW])
            in1 = src_tile[:, NDCOL + STRW:NDCOL + STRW + 1].copy()
            in1.ap[1] = [VW, 2]
            in1.ap.append([1, STRW])
            o = out_tile[:, at:at + 1].copy()
            o.ap[1] = [W, 2]
            o.ap.append([1, STRW])
            nc.vector.tensor_tensor(out=o, in0=in0, in1=in1, op=OP.subtract)

        chunks = []
        for c in range(NCH):
            d0 = 1 + 24 * c
            nd = 24 if c < 7 else 23
            wc = nd * NROW + (STRW if c == 7 else 0)
            chunks.append((c, d0, nd, wc))

        lg_p, bv_p, cross_p, a12_p, exiv_p, exg_p = {}, {}, {}, {}, {}, {}

        def col_chain(tiles, label):
            # tiny [T,1] dependency chain over a list of tiles
            cur = tiles[0][:, 0:1]
            for i, t in enumerate(tiles[1:]):
                nxt = sg.tile([T, 1], f32, name=f"{label}ch{i}")
                nc.vector.tensor_add(nxt[:], cur, t[:, 0:1])
                cur = nxt[:]
            return cur

        def make_tok(dep_ap, value, label):
            tok = sg.tile([T, 1], f32, name=label)
            nc.vector.tensor_scalar(out=tok[:], in0=dep_ap, scalar1=0.0,
                                    scalar2=float(value), op0=OP.mult, op1=OP.add)
            return tok

        def pass_1a(batch, ln_scale):
            for c, d0, nd, wc in batch:
                DV = wk.tile([T, 2 * W], f16, name=f"DV{c}", tag="DV")
                DZ = wk.tile([T, 2 * W], f16, name=f"DZ{c}", tag="DZ")
                ovdiff(DV, vxy, d0, nd)
                ovdiff(DZ, zxy, d0, nd)
                if c == 7:
                    stripdiff(DV, vxy, nd * NROW)
                    stripdiff(DZ, zxy, nd * NROW)
                q12 = wk.tile([T, 2 * W], f16, name=f"q12_{c}", tag="q12")
                nc.scalar.activation(view2(q12, wc), view2(DV, wc), AF.Square)
                a2 = wk.tile([T, W], f16, name=f"a2_{c}", tag="a2")
                nc.vector.tensor_add(a2[:, :wc], q12[:, 0:wc], q12[:, W:W + wc])
                lg = sg.tile([T, W], f16, name=f"lg{c}", tag=f"lg{c}")
                nc.scalar.activation(lg[:, :wc], a2[:, :wc], AF.Ln,
                                     bias=epscol[:],
                                     scale=(1.0 if ln_scale is None else ln_scale[:]))
                bpq = wk.tile([T, 2 * W], f16, name=f"bpq_{c}", tag="bpq")
                nc.vector.tensor_tensor(out=view2(bpq, wc), in0=view2(DZ, wc),
                                        in1=view2(DV, wc), op=OP.mult)
                bv = sg.tile([T, W], f16, name=f"bv{c}", tag=f"bv{c}")
                nc.vector.tensor_add(bv[:, :wc], bpq[:, 0:wc], bpq[:, W:W + wc])
                dvs = DV[:, W:W + 1].copy()
                dvs.ap[1] = [-W, 2]
                dvs.ap.append([1, wc])
                c12 = wk.tile([T, 2 * W], f16, name=f"c12_{c}", tag="c12")
                nc.vector.tensor_tensor(out=view2(c12, wc), in0=view2(DZ, wc),
                                        in1=dvs, op=OP.mult)
                cross = sg.tile([T, W], f16, name=f"cross{c}", tag=f"cross{c}")
                nc.vector.tensor_sub(cross[:, :wc], c12[:, 0:wc], c12[:, W:W + wc])
                lg_p[c], bv_p[c], cross_p[c] = lg, bv, cross

        def pass_1b(batch, halftok, neghalftok):
            for c, d0, nd, wc in batch:
                lg, bv, cross = lg_p[c], bv_p[c], cross_p[c]
                adt = wk.tile([T, W], f16, name=f"adt_{c}", tag="adt")
                nc.scalar.activation(adt[:, :wc], lg[:, :wc], AF.Exp,
                                     scale=halftok[:], bias=lndtcol[:])
                inva = wk.tile([T, W], f16, name=f"inva_{c}", tag="inva")
                nc.scalar.activation(inva[:, :wc], lg[:, :wc], AF.Exp,
                                     scale=neghalftok[:])
                a12 = sg.tile([T, 2 * W], f16, name=f"a12_{c}", tag=f"a12_{c}")
                nc.vector.tensor_tensor(out=a12[:, W:W + wc], in0=bv[:, :wc],
                                        in1=inva[:, :wc], op=OP.mult)
                nc.vector.tensor_add(a12[:, 0:wc], adt[:, :wc], a12[:, W:W + wc])
                u = wk.tile([T, W], f16, name=f"u_{c}", tag="u")
                nc.vector.tensor_tensor(out=u[:, :wc], in0=cross[:, :wc],
                                        in1=inva[:, :wc], op=OP.mult)
                usq = wk.tile([T, W], f16, name=f"usq_{c}", tag="usq")
                nc.vector.tensor_tensor(out=usq[:, :wc], in0=u[:, :wc],
                                        in1=u[:, :wc], op=OP.mult)
                exg = wk.tile([T, W], f16, name=f"exg_{c}", tag="exg")
                nc.scalar.activation(exg[:, :wc], usq[:, :wc], AF.Exp,
                                     scale=-1.0, bias=bln[:])
                exiv = sg.tile([T, W], f16, name=f"exiv_{c}", tag=f"exiv_{c}")
                nc.vector.tensor_tensor(out=exiv[:, :wc], in0=exg[:, :wc],
                                        in1=inva[:, :wc], op=OP.mult)
                a12_p[c], exiv_p[c], exg_p[c] = a12, exiv, exg

        batchA, batchB = chunks[:4], chunks[4:]
        cols = sg.tile([T, NCH], f32, name="cols")
        e12_p = {}

        def pass_p3(batch, zerotok):
            for c, d0, nd, wc in batch:
                a12, exiv = a12_p[c], exiv_p[c]
                e12 = wk.tile([T, 2 * W], f16, name=f"e12_{c}", tag="e12")
                nc.scalar.activation(view2(e12, wc), view2(a12, wc), AF.Erf,
                                     bias=zerotok[:])
                ed = wk.tile([T, W], f16, name=f"ed_{c}", tag="ed")
                nc.vector.tensor_sub(ed[:, :wc], e12[:, 0:wc], e12[:, W:W + wc])
                t4 = wk.tile([T, W], f16, name=f"t4_{c}", tag="t4")
                nc.vector.scalar_tensor_tensor(out=t4[:, :wc], in0=ed[:, :wc],
                                               scalar=1.0, in1=exiv[:, :wc],
                                               op0=OP.mult, op1=OP.mult,
                                               accum_out=cols[:, c:c + 1])
                e12_p[c] = e12

        pass_1a(batchA, None)
        depA = col_chain([lg_p[c] for c, *_ in batchA], "lgA")
        halfA = make_tok(depA, 0.5, "halfA")
        neghalfA = make_tok(depA, -0.5, "neghalfA")
        pass_1b(batchA, halfA, neghalfA)
        depEA = col_chain([exg_p[c] for c, *_ in batchA], "exA")
        zerotokA = make_tok(depEA, 0.0, "zerotokA")
        pass_p3(batchA, zerotokA)
        depP3A = col_chain([e12_p[c] for c, *_ in batchA], "p3A")
        onetokB = make_tok(depP3A, 1.0, "onetokB")
        # gate batch B's Ln behind batch A's erf phase via the Ln scale slot
        pass_1a(batchB, onetokB)
        depB = col_chain([lg_p[c] for c, *_ in batchB], "lgB")
        halfB = make_tok(depB, 0.5, "halfB")
        neghalfB = make_tok(depB, -0.5, "neghalfB")
        pass_1b(batchB, halfB, neghalfB)
        depEB = col_chain([exg_p[c] for c, *_ in batchB], "exB")
        zerotokB = make_tok(depEB, 0.0, "zerotokB")
        pass_p3(batchB, zerotokB)

        colsum = sg.tile([T, 1], f32, name="colsum")
        nc.vector.reduce_sum(out=colsum[:], in_=cols[:], axis=AX.X)
        s_ps = ps.tile([1, 1], f32, name="s_ps")
        nc.tensor.matmul(s_ps[:], colsum[:], ones[:])
        out_sb = sg.tile([1, 2], f32, name="out_sb")
        nc.vector.tensor_copy(out_sb[:, 0:1], ev_s[:])
        nc.vector.tensor_copy(out_sb[:, 1:2], s_ps[:])
        nc.sync.dma_start(out=out_d[:], in_=out_sb[:])

    nc.finalize()
    return nc


def _host_prep(data, t0, tn, z0, v0, beta):
    dt = float(tn - t0) / T
    v0x = np.ascontiguousarray(v0[:, 0, :])
    v0y = np.ascontiguousarray(v0[:, 1, :])
    z0x, z0y = z0[:, 0], z0[:, 1]
    # step-start positions, fp32 on host: z[n, t] = z0 + dt * cumsum(v)[t-1]
    zsx = z0x[:, None] + dt * np.concatenate(
        [np.zeros((N, 1), np.float32), np.cumsum(v0x, axis=1)[:, :-1]], axis=1)
    zsy = z0y[:, None] + dt * np.concatenate(
        [np.zeros((N, 1), np.float32), np.cumsum(v0y, axis=1)[:, :-1]], axis=1)

    times = data[:, 2]
    idx_f = np.floor(times / dt)
    idx = np.where(idx_f < T, idx_f, idx_f - 1.0).astype(np.int32)
    rem = (times - idx_f * dt).astype(np.float32)
    i_idx = np.floor(data[:, 0]).astype(np.int32)
    j_idx = np.floor(data[:, 1]).astype(np.int32)

    in_maps = []
    for c in range(NC):
        rot = (48 * c + np.arange(NDCOL)) % N
        if c < 4:
            si = 48 * c + np.arange(STRW)
            sj = si + 192
        else:
            n_ = 48 * (c - 4) + 24 + np.arange(STRW)
            si, sj = n_ + 192, n_
        nodes = np.concatenate([rot, si, sj])  # 288
        m = {
            "vzxy": np.concatenate([v0x[nodes, :].T, v0y[nodes, :].T,
                                    zsx[nodes, :].T, zsy[nodes, :].T], axis=1).astype(np.float16),
            "vnatx": v0x.astype(np.float16), "vnaty": v0y.astype(np.float16),
            "betac": np.ascontiguousarray(beta[:, None], np.float32),
        }
        es = slice(EV_PER * c, EV_PER * (c + 1))
        ii, jj, dd, rr = i_idx[es], j_idx[es], idx[es], rem[es]
        Q = np.zeros((N, EV_PER), np.float16)
        Wm = np.zeros((T, EV_PER), np.float32)
        B = np.zeros((T, EV_PER), np.float32)
        for e in range(EV_PER):
            Q[ii[e], e] += 1.0
            Q[jj[e], e] -= 1.0
            Wm[:dd[e], e] = dt
            Wm[dd[e], e] += rr[e]
            B[dd[e], e] = 1.0
        m["qmat"], m["wmat"], m["bhot"] = Q, Wm, B
        m["dz0x"] = (z0x[ii] - z0x[jj])[None, :].astype(np.float32)
        m["dz0y"] = (z0y[ii] - z0y[jj])[None, :].astype(np.float32)
        in_maps.append({k: np.ascontiguousarray(v) for k, v in m.items()})
    return dt, in_maps


def _run(inputs, trace=False):
    from concourse.bass_utils import run_bass_kernel_spmd
    data = np.asarray(inputs["data"], np.float32)
    t0 = float(np.asarray(inputs["t0"]))
    tn = float(np.asarray(inputs["tn"]))
    z0 = np.asarray(inputs["z0"], np.float32)
    v0 = np.asarray(inputs["v0"], np.float32)
    beta = np.asarray(inputs["beta"], np.float32)

    dt, in_maps = _host_prep(data, t0, tn, z0, v0, beta)
    if dt not in _CACHE:
        _CACHE[dt] = _build_program(dt)
    nc = _CACHE[dt]
    res = run_bass_kernel_spmd(nc, in_maps, core_ids=list(range(NC)), trace=trace)
    ev = sum(float(res.results[c]["out"][0, 0]) for c in range(NC))
    S = sum(float(res.results[c]["out"][0, 1]) for c in range(NC))
    return np.array(np.float32(ev - S)), res


def kernel(**inputs):
    out, _ = _run(inputs, trace=False)
    return out
